# revision 5
# baseline (speedup 1.0000x reference)
import numpy as np

B = 8
SEQ = 4096
D = 1024
N_BASE = 10000.0
N_CORES = 8
SPC = SEQ // N_CORES  # seq rows per core
JT = SPC // 128       # 128-row chunks per core

_CACHE = {}


def _compute_pe() -> np.ndarray:
    """Mirror of the reference _pos_encoding (default jax backend, f32)."""
    import jax
    import jax.numpy as jnp

    pos = jnp.arange(SEQ, dtype=jnp.float32)[:, None]
    i = jnp.arange(D // 2, dtype=jnp.float32)
    denom = jnp.power(jnp.float32(N_BASE), 2.0 * i / jnp.float32(D))
    ang = pos / denom
    pe = jnp.stack([jnp.sin(ang), jnp.cos(ang)], axis=-1).reshape(SEQ, D)
    return np.asarray(jax.device_get(pe), dtype=np.float32)


def _build_program():
    import concourse.bacc as bacc
    import concourse.mybir as mybir
    import concourse.tile as tile

    nc = bacc.Bacc("TRN2")
    f32 = mybir.dt.float32
    x_in = nc.declare_dram_parameter("x", [B * SPC, D], f32, isOutput=False)
    pe_in = nc.declare_dram_parameter("pe", [SPC, D], f32, isOutput=False)
    y_out = nc.declare_dram_parameter("y", [B * SPC, D], f32, isOutput=True)

    with tile.TileContext(nc) as tc:
        with (
            tc.tile_pool(name="pe_pool", bufs=1) as pe_pool,
            tc.tile_pool(name="x_pool", bufs=6) as x_pool,
        ):
            pe_t = pe_pool.tile([128, JT, D], f32)
            for j in range(JT):
                nc.sync.dma_start(
                    out=pe_t[:, j, :],
                    in_=pe_in[j * 128 : (j + 1) * 128, :],
                )
            for b in range(B):
                for j in range(JT):
                    r0 = b * SPC + j * 128
                    xt = x_pool.tile([128, D], f32)
                    nc.sync.dma_start(out=xt[:], in_=x_in[r0 : r0 + 128, :])
                    nc.vector.tensor_add(xt[:], xt[:], pe_t[:, j, :])
                    nc.sync.dma_start(out=y_out[r0 : r0 + 128, :], in_=xt[:])
    if not nc.is_finalized():
        nc.finalize()
    return nc


def _get_state():
    if "nc" not in _CACHE:
        _CACHE["nc"] = _build_program()
        _CACHE["pe"] = _compute_pe()
    return _CACHE["nc"], _CACHE["pe"]


def kernel(x, seq_len=None, **_):
    from concourse.bass_utils import run_bass_kernel_spmd

    x = np.asarray(x, dtype=np.float32)
    assert x.shape == (B, SEQ, D)
    if seq_len is not None:
        assert int(np.asarray(seq_len)) == SEQ

    nc, pe = _get_state()
    in_maps = []
    for c in range(N_CORES):
        xs = np.ascontiguousarray(
            x[:, c * SPC : (c + 1) * SPC, :]
        ).reshape(B * SPC, D)
        pes = np.ascontiguousarray(pe[c * SPC : (c + 1) * SPC, :])
        in_maps.append({"x": xs, "pe": pes})

    res = run_bass_kernel_spmd(nc, in_maps, list(range(N_CORES))).results

    out = np.empty((B, SEQ, D), dtype=np.float32)
    for c in range(N_CORES):
        out[:, c * SPC : (c + 1) * SPC, :] = res[c]["y"].reshape(B, SPC, D)
    return out


# revision 6
# speedup vs baseline: 1.0509x; 1.0509x over previous
import numpy as np

B = 8
SEQ = 4096
D = 1024
N_BASE = 10000.0
N_CORES = 8
SPC = SEQ // N_CORES  # seq rows per core
JT = SPC // 128       # 128-row chunks per core

_CACHE = {}


def _compute_pe() -> np.ndarray:
    """Mirror of the reference _pos_encoding (default jax backend, f32)."""
    import jax
    import jax.numpy as jnp

    pos = jnp.arange(SEQ, dtype=jnp.float32)[:, None]
    i = jnp.arange(D // 2, dtype=jnp.float32)
    denom = jnp.power(jnp.float32(N_BASE), 2.0 * i / jnp.float32(D))
    ang = pos / denom
    pe = jnp.stack([jnp.sin(ang), jnp.cos(ang)], axis=-1).reshape(SEQ, D)
    return np.asarray(jax.device_get(pe), dtype=np.float32)


def _build_program():
    import concourse.bacc as bacc
    import concourse.mybir as mybir
    import concourse.tile as tile

    nc = bacc.Bacc("TRN2")
    f32 = mybir.dt.float32
    x_in = nc.declare_dram_parameter("x", [B * SPC, D], f32, isOutput=False)
    pe_in = nc.declare_dram_parameter("pe", [SPC, D], f32, isOutput=False)
    y_out = nc.declare_dram_parameter("y", [B * SPC, D], f32, isOutput=True)

    with tile.TileContext(nc) as tc:
        with (
            tc.tile_pool(name="pe_pool", bufs=1) as pe_pool,
            tc.tile_pool(name="x_pool", bufs=8) as x_pool,
        ):
            pe_t = pe_pool.tile([128, JT, D], f32)
            nc.sync.dma_start(
                out=pe_t[:],
                in_=pe_in.rearrange("(u p) d -> p u d", p=128),
            )
            for b in range(B):
                xs = x_in[b * SPC : (b + 1) * SPC, :].rearrange(
                    "(u p) d -> p u d", p=128
                )
                ys = y_out[b * SPC : (b + 1) * SPC, :].rearrange(
                    "(u p) d -> p u d", p=128
                )
                xt = x_pool.tile([128, JT, D], f32)
                nc.sync.dma_start(out=xt[:], in_=xs)
                nc.vector.tensor_add(xt[:], xt[:], pe_t[:])
                nc.sync.dma_start(out=ys, in_=xt[:])
    if not nc.is_finalized():
        nc.finalize()
    return nc


def _get_state():
    if "nc" not in _CACHE:
        _CACHE["nc"] = _build_program()
        _CACHE["pe"] = _compute_pe()
    return _CACHE["nc"], _CACHE["pe"]


def kernel(x, seq_len=None, **_):
    from concourse.bass_utils import run_bass_kernel_spmd

    x = np.asarray(x, dtype=np.float32)
    assert x.shape == (B, SEQ, D)
    if seq_len is not None:
        assert int(np.asarray(seq_len)) == SEQ

    nc, pe = _get_state()
    in_maps = []
    for c in range(N_CORES):
        xs = np.ascontiguousarray(
            x[:, c * SPC : (c + 1) * SPC, :]
        ).reshape(B * SPC, D)
        pes = np.ascontiguousarray(pe[c * SPC : (c + 1) * SPC, :])
        in_maps.append({"x": xs, "pe": pes})

    res = run_bass_kernel_spmd(nc, in_maps, list(range(N_CORES))).results

    out = np.empty((B, SEQ, D), dtype=np.float32)
    for c in range(N_CORES):
        out[:, c * SPC : (c + 1) * SPC, :] = res[c]["y"].reshape(B, SPC, D)
    return out


# revision 8
# speedup vs baseline: 1.2389x; 1.1789x over previous
import numpy as np

B = 8
SEQ = 4096
D = 1024
N_BASE = 10000.0
N_CORES = 8
SPC = SEQ // N_CORES  # seq rows per core
JT = SPC // 128       # 128-row chunks per core

_CACHE = {}


def _compute_pe() -> np.ndarray:
    """Mirror of the reference _pos_encoding (default jax backend, f32)."""
    import jax
    import jax.numpy as jnp

    pos = jnp.arange(SEQ, dtype=jnp.float32)[:, None]
    i = jnp.arange(D // 2, dtype=jnp.float32)
    denom = jnp.power(jnp.float32(N_BASE), 2.0 * i / jnp.float32(D))
    ang = pos / denom
    pe = jnp.stack([jnp.sin(ang), jnp.cos(ang)], axis=-1).reshape(SEQ, D)
    return np.asarray(jax.device_get(pe), dtype=np.float32)


def _build_program():
    import concourse.bacc as bacc
    import concourse.mybir as mybir
    import concourse.tile as tile

    nc = bacc.Bacc("TRN2")
    f32 = mybir.dt.float32
    x_in = nc.declare_dram_parameter("x", [B * SPC, D], f32, isOutput=False)
    pe_in = nc.declare_dram_parameter("pe", [SPC, D], f32, isOutput=False)
    y_out = nc.declare_dram_parameter("y", [B * SPC, D], f32, isOutput=True)

    with tile.TileContext(nc) as tc:
        with (
            tc.tile_pool(name="pe_pool", bufs=1) as pe_pool,
            tc.tile_pool(name="x_pool", bufs=8) as x_pool,
        ):
            pe_t = pe_pool.tile([128, JT, D], f32)
            nc.sync.dma_start(
                out=pe_t[:],
                in_=pe_in.rearrange("(u p) d -> p u d", p=128),
            )
            xts = []
            for b in range(B):
                xs = x_in[b * SPC : (b + 1) * SPC, :].rearrange(
                    "(u p) d -> p u d", p=128
                )
                xt = x_pool.tile([128, JT, D], f32)
                nc.sync.dma_start(out=xt[:], in_=xs)
                xts.append(xt)
            for b in range(B):
                ys = y_out[b * SPC : (b + 1) * SPC, :].rearrange(
                    "(u p) d -> p u d", p=128
                )
                nc.vector.tensor_add(xts[b][:], xts[b][:], pe_t[:])
                nc.scalar.dma_start(out=ys, in_=xts[b][:])
    if not nc.is_finalized():
        nc.finalize()
    return nc


def _get_state():
    if "nc" not in _CACHE:
        _CACHE["nc"] = _build_program()
        _CACHE["pe"] = _compute_pe()
    return _CACHE["nc"], _CACHE["pe"]


def kernel(x, seq_len=None, **_):
    from concourse.bass_utils import run_bass_kernel_spmd

    x = np.asarray(x, dtype=np.float32)
    assert x.shape == (B, SEQ, D)
    if seq_len is not None:
        assert int(np.asarray(seq_len)) == SEQ

    nc, pe = _get_state()
    in_maps = []
    for c in range(N_CORES):
        xs = np.ascontiguousarray(
            x[:, c * SPC : (c + 1) * SPC, :]
        ).reshape(B * SPC, D)
        pes = np.ascontiguousarray(pe[c * SPC : (c + 1) * SPC, :])
        in_maps.append({"x": xs, "pe": pes})

    res = run_bass_kernel_spmd(nc, in_maps, list(range(N_CORES))).results

    out = np.empty((B, SEQ, D), dtype=np.float32)
    for c in range(N_CORES):
        out[:, c * SPC : (c + 1) * SPC, :] = res[c]["y"].reshape(B, SPC, D)
    return out
